# revision 121
# baseline (speedup 1.0000x reference)
"""Trainium2 Bass kernel for nn_BEM_50002009260181.

Module (B=4, L=1024, D=768, F=32):
    AKey   = tanh(A @ W_aup1.T + b_aup1)          (B,L,D)
    AValue = tan (A @ W_aup2.T + b_aup2)          (B,L,D)
    VKey   = tanh(V @ W_vup1.T + b_vup1)          (B,L,D)
    VValue = tanh(V @ W_vup2.T + b_vup2)          (B,L,D)
    TAQ    = tanh(T * (A @ w_a.T) + b_a)          (B,L,D)
    TVQ    = tanh(T * (V @ w_v.T) + b_v)          (B,L,D)
    ta     = softmax_L(sum_d TAQ*VKey)            (B,L)
    tv     = softmax_L(sum_d TVQ*AKey)            (B,L)
    out    = (AValue * ta[...,None], VValue * tv[...,None])

Sharding: 8 cores = (batch b, stream s).  Even cores own the a-stream of
their batch (full-L s_ta softmax + out_a), odd cores the v-stream.  The two
streams are structurally identical up to which of A/V and which weights
feed each op, so a SINGLE SPMD program serves all cores with per-core
input binding:

    q     = M @ w_q               M = A (even) / V (odd)     [fp16 mm]
    tq    = tanh(q*T + b_q)       = TAQ / TVQ                [ACT, T fp16]
    key   = tanh(O @ W_key + b)   O = V/A -> VKey / AKey     [fp16 mm]
    s     = sum_d tq*key          = s_ta / s_tv              [DVE STT]
    tn    = softmax(s)            = ta / tv
    x     = M @ W_x + b  (fp32)   = x_a2 / x_v2
    out_sin  = sin(w)/sin(pi/2-|w|) * tn   w = wrap_pi(x)    -> out_a (even)
    y     = M @ W_y + b  (fp16)
    out_tanh = tanh(y) * tn                                  -> out_v (odd)

Each core computes BOTH value paths (one discarded by the host); the waste
is the price of a uniform program and far cheaper than the baseline's
duplicated full-L score phase (ACT: 27 wide ops vs 44).

tan period-pi trick: max|x| = 3.70 < pi/2 + pi, so w = wrap(x, +-pi/2,
period pi) needs at most one subtraction and tan(x) = tan(w) exactly;
cos(w) = sin(w + pi/2) keeps the Sin argument inside [0, pi], and
sin''(pi) = 0 keeps the pole-side interpolation accurate (pole distance
~2.9e-4 vs argument error ~2e-7).  fp16 everywhere except the
pole-critical x / sin / cos / reciprocal chain (score-path fp16 error
~1.7e-3 << 2e-2 tolerance).

ACT tables: phase-1 tanh + exp + the first half of the vval tanhs live in
`exp_and_others`; one switch to the sin set covers all Sin ops; the
trailing vval half rides one switch back while the sin-phase DVE/DMA
pipeline drains underneath (3 loads total, the first hidden in warmup).

PSUM: keys/y pairs rotate one 3-bank buffer (bank-boundary matmul splits
512|256|256|512); x tiles rotate two 2-bank buffers so the PE never
head-of-line blocks on an ACT read.  Startup DMAs split across the sync
(HWDGE) and Pool (SWDGE) queues, and the PE pstate ramp is kept warm
through the DMA window so the fp32 x matmuls run at full rate.
"""

import numpy as np

B, L, D, F = 4, 1024, 768, 32
NCORES = 8
LT = 128          # l-tile size (partition dim)
NT = L // LT      # 8 l-tiles
NP = NT // 2      # 4 tile-pairs
K1 = F + 1        # contraction with bias row

PI = float(np.pi)
PIO2 = float(np.float32(np.pi / 2))

_CACHE = {}


def _build():
    if "nc" in _CACHE:
        return _CACHE["nc"]

    import concourse.bacc as bacc
    from concourse import bass_isa
    import concourse.tile as tile
    import concourse.mybir as mybir
    from concourse.tile import add_dep_helper

    F32 = mybir.dt.float32
    F16 = mybir.dt.float16
    AF = mybir.ActivationFunctionType
    ALU = mybir.AluOpType

    nc = bacc.Bacc()

    # ---- DRAM I/O (per-core binding; M/O/W differ by stream parity) ----
    # fp16 operands ride one packed tensor: [wq | mq | wkey | ok | wy]
    C_WQ, C_MQ, C_WK, C_OK, C_WY = 0, 1, 1 + L, 1 + L + D, 1 + 2 * L + D
    C16 = C_WY + D
    d_t16 = nc.dram_tensor("t16", [L, D], F16, kind="ExternalInput")
    d_p16 = nc.dram_tensor("pack16", [K1, C16], F16, kind="ExternalInput")
    d_m32 = nc.dram_tensor("m32_pack", [K1, L], F32, kind="ExternalInput")
    d_wx = nc.dram_tensor("w_x", [K1, D], F32, kind="ExternalInput")
    d_bq = nc.dram_tensor("b_q", [LT, 1], F32, kind="ExternalInput")
    d_os = nc.dram_tensor("out_sin", [L, D], F16, kind="ExternalOutput")
    d_ot = nc.dram_tensor("out_tanh", [L, D], F16, kind="ExternalOutput")

    t_view = d_t16.rearrange("(n p) d -> p n d", p=LT)    # [128, 8, 768]
    os_view = d_os.rearrange("(n p) d -> p n d", p=LT)
    ot_view = d_ot.rearrange("(n p) d -> p n d", p=LT)

    # psum-bank-respecting matmul splits for a [LT, 1536] pair tile
    def pair_mms(dst, lhs_tile, lhs_c0, rhs_tile, rhs_c0, h):
        cuts = (0, 512, D) if h == 0 else (D, D + 256, 2 * D)
        for c0, c1 in zip(cuts[:-1], cuts[1:]):
            nc.tensor.matmul(
                dst[:, c0:c1],
                lhs_tile[0:K1, lhs_c0 : lhs_c0 + LT],
                rhs_tile[0:K1, rhs_c0 + c0 - h * D : rhs_c0 + c1 - h * D],
                start=True, stop=True)

    with tile.TileContext(nc) as tc:
        with (
            tc.tile_pool(name="consts", bufs=1) as consts,
            tc.tile_pool(name="tq", bufs=4) as tqp,
            tc.tile_pool(name="kk", bufs=2) as kkp,
            tc.tile_pool(name="scr", bufs=2) as scrp,
            tc.tile_pool(name="vv", bufs=8) as vvp,
            tc.tile_pool(name="snp", bufs=3) as snp,
            tc.tile_pool(name="rcp", bufs=3) as rcp,
            tc.tile_pool(name="outp", bufs=2) as outp,
            tc.tile_pool(name="ps", bufs=1, space="PSUM") as ps,
        ):
            # ---- startup DMAs, ordered by first-consumer time: the t16
            # tiles gate the tq stream, the fp32 x operands have slack.
            # Late t16 tiles dispatch from the Pool queue (SWDGE) so the
            # weight packs on the sync queue don't serialize behind them ----
            p16 = consts.tile([K1, C16], F16, tag="p16")
            nc.sync.dma_start(out=p16[:, C_WQ:C_WK], in_=d_p16[:, C_WQ:C_WK])
            sb_bq = consts.tile([LT, 1], F32, tag="sb_bq")
            nc.sync.dma_start(out=sb_bq[:], in_=d_bq[:])
            t16 = consts.tile([LT, NT, D], F16, tag="t16")
            nc.sync.dma_start(out=t16[:, 0:1, :], in_=t_view[:, 0:1, :])
            nc.sync.dma_start(out=t16[:, 1:2, :], in_=t_view[:, 1:2, :])
            nc.gpsimd.dma_start(out=t16[:, 2:4, :], in_=t_view[:, 2:4, :])
            nc.sync.dma_start(out=p16[:, C_WK:C_WY], in_=d_p16[:, C_WK:C_WY])
            nc.gpsimd.dma_start(out=t16[:, 4:6, :], in_=t_view[:, 4:6, :])
            sb_wx = consts.tile([K1, D], F32, tag="sb_wx")
            nc.sync.dma_start(out=sb_wx[:], in_=d_wx[:])
            nc.gpsimd.dma_start(out=t16[:, 6:NT, :], in_=t_view[:, 6:NT, :])
            sb_m32 = consts.tile([K1, L], F32, tag="sb_m32")
            nc.sync.dma_start(out=sb_m32[:], in_=d_m32[:])
            nc.sync.dma_start(out=p16[:, C_WY:C16], in_=d_p16[:, C_WY:C16])

            # ---- warmups: PE pstate ramp (memsets on the idle DVE queue —
            # the Pool queue is busy with SWDGE descriptor generation) ----
            dmy = consts.tile([F, 64], F16, tag="dmy")
            nc.vector.memset(dmy[:], 0.0)
            ps_d = ps.tile([64, 64], F32, tag="psq", bufs=1, name="ps_d")
            for _k in range(34):
                nc.tensor.matmul(ps_d[:], dmy[:, 0:64], dmy[:, 0:64], start=True, stop=True)
            sb_pio2 = consts.tile([LT, 1], F32, tag="sb_pio2")
            nc.vector.memset(sb_pio2[:], PIO2)

            # ---- q matmuls: q[l] = M[l,:] @ w_q ----
            ps_q = ps.tile([LT, NT], F32, tag="psq", bufs=1)
            for i in range(NT):
                nc.tensor.matmul(
                    ps_q[:, i : i + 1],
                    p16[0:F, C_MQ + i * LT : C_MQ + (i + 1) * LT],
                    p16[0:F, C_WQ : C_WQ + 1],
                    start=True, stop=True,
                )
            sb_q = consts.tile([LT, NT], F32, tag="sb_q")
            nc.vector.tensor_copy(out=sb_q[:], in_=ps_q[:])
            # keep PE continuously busy until the key-path DMAs land, else the
            # pstate ramp resets and the fp32 x matmuls run at half speed.
            # These bridge dummies use the pska rotation (not ps_q's buffer) so
            # they cannot entangle the sb_q copy's dependencies.
            ps_d2 = ps.tile([64, 64], F32, tag="pska", bufs=1, name="ps_d2")
            for _k in range(24):
                nc.tensor.matmul(ps_d2[:], dmy[:, 0:64], dmy[:, 0:64], start=True, stop=True)

            sb_s = consts.tile([LT, NT], F32, tag="sb_s")
            w_all = consts.tile([LT, NT * D], F32, tag="w_all")

            # ---- phase 1: score stream (tanh table), pair-granular ----
            # PSUM: key/y pairs rotate one 3-bank buffer (tag pska); x tiles
            # rotate two 1.5-bank buffers (tag psx) so the PE never
            # head-of-line blocks on an ACT read.
            for p in range(NP):
                i0 = 2 * p
                ps_k = ps.tile([LT, 2 * D], F32, tag="pska", bufs=1, name=f"ps_k{p}")
                for h in range(2):
                    pair_mms(ps_k, p16, C_OK + (i0 + h) * LT, p16, C_WK, h)
                ps_xs = []
                for h in range(2):
                    lsl = slice((i0 + h) * LT, (i0 + h + 1) * LT)
                    ps_x = ps.tile([LT, D], F32, tag="psx", bufs=2, name=f"ps_x{i0 + h}")
                    nc.tensor.matmul(ps_x[:, 0:512], sb_m32[:, lsl],
                                     sb_wx[:, 0:512], start=True, stop=True)
                    nc.tensor.matmul(ps_x[:, 512:D], sb_m32[:, lsl],
                                     sb_wx[:, 512:D], start=True, stop=True)
                    ps_xs.append(ps_x)

                tq0 = tqp.tile([LT, D], F16, tag="tq")
                nc.scalar.activation(out=tq0[:], in_=t16[:, i0, :], func=AF.Tanh,
                                     bias=sb_bq[:], scale=sb_q[:, i0 : i0 + 1])
                tq1 = tqp.tile([LT, D], F16, tag="tq")
                nc.scalar.activation(out=tq1[:], in_=t16[:, i0 + 1, :], func=AF.Tanh,
                                     bias=sb_bq[:], scale=sb_q[:, i0 + 1 : i0 + 2])
                kk = kkp.tile([LT, 2 * D], F16, tag="kk")
                nc.scalar.activation(out=kk[:], in_=ps_k[:], func=AF.Tanh)
                # wraps first on DVE so the x-psum rotation never backs up
                for h in range(2):
                    wsl = slice((i0 + h) * D, (i0 + h + 1) * D)
                    nc.vector.add_range_wrap(out=w_all[:, wsl], in_=ps_xs[h][:],
                                             shift=0.0, bound=PIO2, period=PI)
                for h, tq in ((0, tq0), (1, tq1)):
                    scr = scrp.tile([LT, D], F16, tag="scr")
                    nc.vector.scalar_tensor_tensor(
                        out=scr[:], in0=kk[:, h * D : (h + 1) * D], scalar=1.0,
                        in1=tq[:], op0=ALU.mult, op1=ALU.mult,
                        accum_out=sb_s[:, i0 + h : i0 + h + 1],
                    )

            # ---- softmax + value tanh phase (still exp_and_others table) ----
            vv_insts = []

            def emit_vv(j, tag="psx"):
                """Single-tile vval through the psx rotation; the first one
                rides the pska buffer (free right after the last kk read)
                so it does not wait for the x stream to release psx."""
                ps_y = ps.tile([LT, D], F32, tag=tag,
                               bufs=(1 if tag == "pska" else 2), name=f"ps_y{j}")
                nc.tensor.matmul(ps_y[:, 0:512],
                                 p16[0:K1, C_MQ + j * LT : C_MQ + (j + 1) * LT],
                                 p16[0:K1, C_WY : C_WY + 512],
                                 start=True, stop=True)
                nc.tensor.matmul(ps_y[:, 512:D],
                                 p16[0:K1, C_MQ + j * LT : C_MQ + (j + 1) * LT],
                                 p16[0:K1, C_WY + 512 : C_WY + D],
                                 start=True, stop=True)
                vv = vvp.tile([LT, D], F16, tag="vv")
                vv_insts.append(nc.scalar.activation(out=vv[:], in_=ps_y[:], func=AF.Tanh))
                return vv

            def emit_ot(j, vv):
                ot = outp.tile([LT, D], F16, tag="ot", bufs=4)
                inst = nc.vector.tensor_scalar(
                    out=ot[:], in0=vv[:],
                    scalar1=tn[:, j : j + 1], scalar2=None, op0=ALU.mult,
                )
                nc.sync.dma_start(out=ot_view[:, j : j + 1, :], in_=ot[:])
                return inst

            e_t = consts.tile([LT, NT], F32, tag="e_t")
            rsum = consts.tile([LT, 1], F32, tag="rsum")
            zsum = consts.tile([LT, 1], F32, tag="zsum")
            inv = consts.tile([LT, 1], F32, tag="inv")
            tn = consts.tile([LT, NT], F32, tag="tn")

            # vv tanh needs neither exp nor tn: the first half keeps ACT busy
            # while the softmax chain resolves; the second half runs after the
            # sins (one extra table load) so the sin/recip pipeline starts
            # ~3us earlier and its DVE tail drains under real ACT work.
            NVE = 3   # vvals before the sin phase: only the LAST one gates
            vvs = [emit_vv(j, tag=("pska" if j == 0 else "psx")) for j in range(NVE)]
            exp_inst = nc.scalar.activation(out=e_t[:], in_=sb_s[:], func=AF.Exp,
                                            accum_out=rsum[:])
            nc.gpsimd.partition_all_reduce(zsum[:], rsum[:], channels=LT,
                                           reduce_op=bass_isa.ReduceOp.add)
            nc.vector.reciprocal(out=inv[:], in_=zsum[:])
            nc.vector.tensor_scalar(out=tn[:], in0=e_t[:], scalar1=inv[:],
                                    scalar2=None, op0=ALU.mult)
            early_ots = [emit_ot(j, vvs[j]) for j in range(NVE)]

            # ---- sin phase: tan(x)*tn -> out_sin (one table switch) ----
            def sin_block(j0, ntile, defer_os=False):
                """Sin ops for tiles [j0, j0+ntile); per-tile recip + out."""
                wsl = slice(j0 * D, (j0 + ntile) * D)
                sn = snp.tile([LT, ntile * D], F32, tag="sn")
                i1 = nc.scalar.activation(out=sn[:], in_=w_all[:, wsl], func=AF.Sin)
                # cos(w) = sin(w + pi/2); arg in [0, pi] stays in table range,
                # and sin''(pi) = 0 keeps the pole-side interpolation accurate
                cs = snp.tile([LT, ntile * D], F32, tag="cs")
                i2 = nc.scalar.activation(out=cs[:], in_=w_all[:, wsl],
                                          func=AF.Sin, bias=sb_pio2[:])
                for ins in (i1, i2):
                    add_dep_helper(ins.ins, exp_inst.ins, sync=False,
                                   reason="sin after exp (ACT table set)")
                    add_dep_helper(ins.ins, vv_insts[-1].ins, sync=False,
                                   reason="sin after last vval tanh (ACT table set)")
                def emit_os(t):
                    j = j0 + t
                    rc = rcp.tile([LT, D], F32, tag="rc")
                    rci = nc.vector.reciprocal_approx_fast(out=rc[:], in_=cs[:, t * D : (t + 1) * D])
                    if j0 == 0 and t == 0:
                        # keep the cheap early-out scales (and their DMAs)
                        # ahead of the sin-phase DVE queue
                        for oti in early_ots:
                            add_dep_helper(rci.ins, oti.ins, sync=False,
                                           reason="first recip after early ot scales")
                    os_t = outp.tile([LT, D], F16, tag="os")
                    nc.vector.scalar_tensor_tensor(
                        out=os_t[:], in0=sn[:, t * D : (t + 1) * D],
                        scalar=tn[:, j : j + 1], in1=rc[:],
                        op0=ALU.mult, op1=ALU.mult,
                    )
                    nc.sync.dma_start(out=os_view[:, j : j + 1, :], in_=os_t[:])

                if not defer_os:
                    for t in range(ntile):
                        emit_os(t)
                return i2, emit_os

            sin_block(0, 2)
            sin_block(2, 2)
            sin_block(4, 2)
            last_sin, last_os = sin_block(6, 2, defer_os=True)

            # trailing vval half: tanh rides one table switch back while the
            # sin-phase DVE/DMA pipeline drains underneath.  The last sin
            # block's recip/scale chains interleave between the two output
            # batches so neither queue idles in the drain.
            otl = outp.tile([LT, NT - NVE, D], F16, tag="otl")

            def emit_otl(j):
                vv = emit_vv(j)
                add_dep_helper(vv_insts[-1].ins, last_sin.ins, sync=False,
                               reason="late vval after last sin (table order)")
                nc.vector.tensor_scalar(
                    out=otl[:, j - NVE, :], in0=vv[:],
                    scalar1=tn[:, j : j + 1], scalar2=None, op0=ALU.mult,
                )

            emit_otl(3)
            last_os(0)
            emit_otl(4)
            nc.sync.dma_start(out=ot_view[:, 3:5, :], in_=otl[:, 0:2, :])
            last_os(1)
            emit_otl(5)
            emit_otl(6)
            nc.sync.dma_start(out=ot_view[:, 5:7, :], in_=otl[:, 2:4, :])
            emit_otl(7)
            nc.sync.dma_start(out=ot_view[:, 7:NT, :], in_=otl[:, 4:5, :])

    nc.finalize()
    _CACHE["nc"] = nc
    return nc


def _prep_in_maps(T, A, V, w_a, b_a, w_v, b_v,
                  W_aup1, b_aup1, W_aup2, b_aup2,
                  W_vup1, b_vup1, W_vup2, b_vup2):
    f32, f16 = np.float32, np.float16
    T = np.asarray(T, f32)
    A = np.asarray(A, f32)
    V = np.asarray(V, f32)

    def aug(W, b, dt):
        return np.ascontiguousarray(
            np.concatenate([np.asarray(W, f32).T, np.asarray(b, f32)[None, :]],
                           axis=0).astype(dt))

    def pack(M, dt):
        out = np.ones((K1, L), dt)
        out[0:F] = M.T.astype(dt)
        return out

    in_maps = []
    for c in range(NCORES):
        b, s = divmod(c, 2)
        M, O = (A[b], V[b]) if s == 0 else (V[b], A[b])
        wq, bq = (w_a, b_a) if s == 0 else (w_v, b_v)
        Wk, bk = (W_vup1, b_vup1) if s == 0 else (W_aup1, b_aup1)
        W2, b2 = (W_aup2, b_aup2) if s == 0 else (W_vup2, b_vup2)
        bq_t = np.full((LT, 1), np.asarray(bq, f32).reshape(()), f32)
        # pack16 cols: [wq | mq | wkey | ok | wy]
        p16 = np.ones((K1, 1 + 2 * L + 2 * D), f16)
        p16[0:F, 0:1] = np.asarray(wq, f32).reshape(F, 1).astype(f16)
        p16[:, 1 : 1 + L] = pack(M, f16)
        p16[:, 1 + L : 1 + L + D] = aug(Wk, bk, f16)
        p16[:, 1 + L + D : 1 + 2 * L + D] = pack(O, f16)
        p16[:, 1 + 2 * L + D :] = aug(W2, b2, f16)
        in_maps.append({
            "t16": np.ascontiguousarray(T[b].astype(f16)),
            "pack16": p16,
            "m32_pack": pack(M, f32),
            "w_x": aug(W2, b2, f32),
            "b_q": bq_t,
        })
    return in_maps


def kernel(**inputs):
    from concourse.bass_utils import run_bass_kernel_spmd

    nc = _build()
    in_maps = _prep_in_maps(**inputs)
    res = run_bass_kernel_spmd(nc, in_maps, core_ids=list(range(NCORES)))

    out_a = np.empty((B, L, D), np.float32)
    out_v = np.empty((B, L, D), np.float32)
    for b in range(B):
        out_a[b] = res.results[2 * b]["out_sin"].astype(np.float32)
        out_v[b] = res.results[2 * b + 1]["out_tanh"].astype(np.float32)
    return out_a, out_v
